# revision 1
# baseline (speedup 1.0000x reference)
"""Trainium2 Bass kernel for multi-head self-attention.

Problem: B=4, S=2048, D=1024, H=16 heads (HD=64), fp32 I/O.
  qkv = x @ w_qkv + b_qkv ; attention(softmax(q k^T / 8) v) ; out @ w_out + b_out

Sharding over 8 NeuronCores: core c handles batch b=c//2 and heads
half=c%2 (8 heads each).  Each core computes a partial output
(its heads' contribution to out[b] @ w_out); the host sums the two
partials per batch and adds the constant bias terms.

Matmul operands are fp16 (DT) by default; scores are accumulated in
fp32 PSUM, exp() runs on ScalarE in fp32 with the 1/sqrt(HD) scale
folded in, and softmax is computed unnormalized with the row-sum from a
fused ones-column in the PV matmul, normalized at the [64, S] stage.
"""

import contextlib
import numpy as np

import concourse.bacc as bacc
import concourse.tile as tile
from concourse.tile_rust import add_dep_helper
from concourse import mybir
from concourse.bass_utils import run_bass_kernel_spmd

B, S, D, H, HD = 4, 2048, 1024, 16, 64
NCORES = 8
NH = 8            # heads per core
QF = 512          # q features per core (= NH * HD), same for k and v
PC = 512          # position chunk (psum bank, fp32)
NPC = S // PC     # 4 position chunks
KT = S // 128     # 16 key-position tiles
DC = D // 128     # 8 contraction chunks
FT_QK = (2 * QF) // 128   # 8 feature tiles of qk
FT_AT = QF // 128         # 4 feature tiles of attn output

F32R = mybir.dt.float32r
F32 = mybir.dt.float32
F16 = mybir.dt.float16
DT = F16          # matmul operand dtype (F16 or F32R)
DT_NP = np.float16 if DT == F16 else np.float32

_CACHE = {}


def _build(repeat=1):
    nc = bacc.Bacc("TRN2", target_bir_lowering=False, debug=False)

    xT = nc.dram_tensor("xT", [D, S], DT, kind="ExternalInput").ap()
    wqk = nc.dram_tensor("wqk", [D, 2 * QF], DT, kind="ExternalInput").ap()
    wv = nc.dram_tensor("wv", [D, QF], DT, kind="ExternalInput").ap()
    bqk = nc.dram_tensor("bqk", [2 * QF, 1], F32, kind="ExternalInput").ap()
    wo = nc.dram_tensor("wo", [QF, D], DT, kind="ExternalInput").ap()
    sel2 = nc.dram_tensor("sel2", [2, 128], F32R, kind="ExternalInput").ap()
    out_d = nc.dram_tensor("out_partial", [S, D], F32, kind="ExternalOutput").ap()

    with tile.TileContext(nc) as tc:
        with contextlib.ExitStack() as ctx:
            with nc.allow_low_precision(reason="f32r/fp16 intermediates are intentional"):
                token = None
                for _ in range(repeat):
                    token = _emit(nc, tc, ctx, xT, wqk, wv, bqk, wo, sel2, out_d,
                                  token=token)
    nc.compile()
    return nc


def _emit(nc, tc, ctx, xT, wqk, wv, bqk, wo, sel2, out_d, token=None):
    with contextlib.ExitStack() as kctx:
        return _emit_inner(nc, tc, kctx, xT, wqk, wv, bqk, wo, sel2, out_d, token)


def _emit_inner(nc, tc, ctx, xT, wqk, wv, bqk, wo, sel2, out_d, token=None):
    # ---- long-lived tensors -------------------------------------------------
    keep = ctx.enter_context(tc.tile_pool(name="keep", bufs=1))
    qkT = keep.tile([128, FT_QK, S], DT, tag="qkT")            # 32 KB/p (fp16)
    v_sb = keep.tile([128, KT, NH, HD + 1], F16, tag="v_sb")   # 16.3 KB/p
    attn = keep.tile([128, FT_AT, S], DT, tag="attn")          # 16 KB/p (fp16)
    bqk_t = keep.tile([128, FT_QK, 1], F32, tag="bqk")
    sel_t = keep.tile([128, 2, 128], F32R, tag="sel")
    wo_t = keep.tile([128, FT_AT, D], DT, tag="wo_t")          # 8 KB/p (fp16)
    xt = keep.tile([128, DC, S], DT, tag="xt")                 # 32 KB/p (fp16)
    wv_t = keep.tile([128, DC, QF], DT, tag="wv_t")            # 8 KB/p
    E_sb0 = keep.tile([128, KT, 2, PC], F16, tag="E_sb0")      # 32 KB/p
    E_sb1 = keep.tile([128, KT, 2, PC], F16, tag="E_sb1")      # 32 KB/p
    E_bufs = (E_sb0, E_sb1)

    wqk_pool = ctx.enter_context(tc.tile_pool(name="wqk_pool", bufs=2))
    rec_pool = ctx.enter_context(tc.tile_pool(name="rec_pool", bufs=1))
    rb_pool = ctx.enter_context(tc.tile_pool(name="rb_pool", bufs=2))
    stg = ctx.enter_context(tc.tile_pool(name="stg", bufs=2))
    ps = ctx.enter_context(tc.tile_pool(name="ps", bufs=4, space="PSUM"))

    def emit_input_dmas():
        first_dmas = []
        first_dmas.append(nc.sync.dma_start(
            out=bqk_t, in_=bqk.rearrange("(ft p) o -> p ft o", p=128)))
        first_dmas.append(nc.sync.dma_start(
            out=sel_t[0:1, :, :],
            in_=sel2.rearrange("a b -> (a b)")[None, :].rearrange(
                "o (a b) -> o a b", a=2)))
        # x^T arrives position-chunk-major so compute can start early
        for pc in range(NPC):
            for dc in range(DC):
                first_dmas.append(nc.sync.dma_start(
                    out=xt[:, dc, pc * PC:(pc + 1) * PC],
                    in_=xT[dc * 128:(dc + 1) * 128, pc * PC:(pc + 1) * PC]))
        for dc in range(DC):
            first_dmas.append(nc.sync.dma_start(
                out=wv_t[:, dc, :], in_=wv[dc * 128:(dc + 1) * 128, :]))
        for fc in range(FT_AT):
            first_dmas.append(nc.sync.dma_start(
                out=wo_t[:, fc, :], in_=wo[fc * 128:(fc + 1) * 128, :]))
        if token is not None:
            for i in first_dmas:
                add_dep_helper(token.ins, i.ins, sync=True,
                               reason="serialize benchmark repeats")

    def b1_dma(ft):
        """fetch one feature tile of the qk weights."""
        wqk_t = wqk_pool.tile([128, DC, 128], DT, tag="wqk_t", name=f"wqk{ft}")
        for dc in range(DC):
            i = nc.sync.dma_start(
                out=wqk_t[:, dc, :],
                in_=wqk[dc * 128:(dc + 1) * 128, ft * 128:(ft + 1) * 128])
            if token is not None:
                add_dep_helper(token.ins, i.ins, sync=True,
                               reason="serialize benchmark repeats")
        return wqk_t

    def b1_mm(wqk_t, ft, pc2):
        """qkT[f, s] = sum_d wqk[d, f] * xT[d, s]  (+ bias), 2 pos chunks."""
        qp = ps.tile([128, 2, PC], F32, tag="ps", name=f"qkps{ft}_{pc2}")
        for dc in range(DC):
            for j in range(2):
                pc = pc2 * 2 + j
                nc.tensor.matmul(
                    qp[:, j, :], wqk_t[:, dc, :],
                    xt[:, dc, pc * PC:(pc + 1) * PC],
                    start=(dc == 0), stop=(dc == DC - 1))
        for j in range(2):
            pc = pc2 * 2 + j
            nc.vector.tensor_scalar_add(
                out=qkT[:, ft, pc * PC:(pc + 1) * PC],
                in0=qp[:, j, :], scalar1=bqk_t[:, ft, :])

    def b1_ft(ft):
        wqk_t = b1_dma(ft)
        for pc2 in range(NPC // 2):
            b1_mm(wqk_t, ft, pc2)

    def b2_block(st2):
        """v[s, f] natural layout (+ ones column), two position tiles."""
        vp = ps.tile([128, 2, PC], F32, tag="ps", name=f"vps{st2}")
        for dc in range(DC):
            for j in range(2):
                st = st2 * 2 + j
                nc.tensor.matmul(
                    vp[:, j, :], xt[:, dc, st * 128:(st + 1) * 128], wv_t[:, dc, :],
                    start=(dc == 0), stop=(dc == DC - 1))
        for j in range(2):
            st = st2 * 2 + j
            nc.vector.tensor_copy(
                out=v_sb[:, st, :, 0:HD],
                in_=vp[:, j, :].rearrange("p (h d) -> p h d", h=NH))

    def emit_sc(E_sb, pp, qc, kt):
        """scores^T matmuls + exp for one kt tile of block (pp, qc)."""
        kft = FT_AT + pp
        qft = pp
        qs = slice(qc * PC, (qc + 1) * PC)
        ks = slice(kt * 128, (kt + 1) * 128)
        sc = ps.tile([128, 2, PC], F32, tag="ps", name=f"sc{pp}_{qc}_{kt}")
        nc.tensor.matmul(
            sc[:, 0, :], qkT[0:64, kft, ks], qkT[0:64, qft, qs],
            start=True, stop=True)
        nc.tensor.matmul(
            sc[:, 1, :], qkT[64:128, kft, ks], qkT[64:128, qft, qs],
            start=True, stop=True)
        nc.scalar.activation(
            out=E_sb[:, kt, :, :], in_=sc,
            func=mybir.ActivationFunctionType.Exp, scale=0.125)

    def emit_pv(E_sb, pv, pp, kt):
        """PV (+ fused row-sum) matmuls for one kt tile of block (pp, qc)."""
        nc.tensor.matmul(
            pv[0:HD + 1, 0, :], v_sb[:, kt, 2 * pp, :], E_sb[:, kt, 0, :],
            start=(kt == 0), stop=(kt == KT - 1))
        nc.tensor.matmul(
            pv[0:HD + 1, 1, :], v_sb[:, kt, 2 * pp + 1, :], E_sb[:, kt, 1, :],
            start=(kt == 0), stop=(kt == KT - 1))

    def emit_norm(pv, pp, qc):
        """normalize: attn[:, pp] = pv[0:64] / rowsum  (both heads)."""
        qs = slice(qc * PC, (qc + 1) * PC)
        rec = rec_pool.tile([128, 2, PC], F32R, tag="rec")
        nc.vector.reciprocal(out=rec[0:1, 0, :], in_=pv[HD:HD + 1, 0, :])
        nc.vector.reciprocal(out=rec[0:1, 1, :], in_=pv[HD:HD + 1, 1, :])
        rb = ps.tile([128, 2, PC], F32, tag="ps", name=f"rb{pp}_{qc}")
        nc.tensor.matmul(rb[:, 0, :], sel_t[0:1, 0, :], rec[0:1, 0, :],
                         start=True, stop=False)
        nc.tensor.matmul(rb[:, 0, :], sel_t[0:1, 1, :], rec[0:1, 1, :],
                         start=False, stop=True)
        rb_sb = rb_pool.tile([128, PC], F32R, tag="rb_sb")
        nc.vector.tensor_copy(out=rb_sb, in_=rb[:, 0, :])
        nc.vector.tensor_mul(
            out=attn[0:64, pp, qs], in0=pv[0:HD, 0, :], in1=rb_sb[0:64, :])
        nc.vector.tensor_mul(
            out=attn[64:128, pp, qs], in0=pv[0:HD, 1, :], in1=rb_sb[64:128, :])

    last_copy = None

    def d_block(qc):
        """output projection for one q-chunk (4 position tiles)."""
        nonlocal last_copy
        for j in range(4):
            st = qc * 4 + j
            ss = slice(st * 128, (st + 1) * 128)
            op = ps.tile([128, 2, PC], F32, tag="ps", name=f"ops{st}")
            for fc in range(FT_AT):
                for n in range(2):
                    nc.tensor.matmul(
                        op[:, n, :], attn[:, fc, ss],
                        wo_t[:, fc, n * PC:(n + 1) * PC],
                        start=(fc == 0), stop=(fc == FT_AT - 1))
            ot = stg.tile([128, D], F32, tag="ot")
            last_copy = nc.vector.tensor_copy(
                out=ot, in_=op.rearrange("p a b -> p (a b)"))
            nc.sync.dma_start(out=out_d[ss, :], in_=ot)

    # ---- emission schedule ---------------------------------------------------
    # Tile directs dependencies by emission order, so every producer must be
    # emitted before its consumer.  Attention blocks form a depth-2 software
    # pipeline at kt granularity: block i's PV matmuls interleave with block
    # i+1's scores+exp, so ScalarE (exp) never starves while TensorE works.
    # Projection feature tiles are emitted as fillers early in each head
    # pair; the output projection for q-chunk qc runs inside the last pair.
    nc.vector.memset(v_sb[:, :, :, HD:HD + 1], 1.0)
    # weight tiles for the first head pair go out before the bulk input,
    # and only the position chunks the first scores need are computed before
    # the first exp, so ScalarE starts as early as possible
    w0 = b1_dma(0)
    w4 = b1_dma(FT_AT + 0)
    emit_input_dmas()
    b1_mm(w0, 0, 0)
    b1_mm(w4, FT_AT + 0, 0)
    # remaining qk weight-tile fetches, in use order
    wts = {0: w0, FT_AT: w4}
    for f in (1, FT_AT + 1, 2, FT_AT + 2, 3, FT_AT + 3):
        wts[f] = b1_dma(f)
    blocks = [(pp, qc) for pp in range(NH // 2) for qc in range(NPC)]
    # spread remaining projection compute, one half-feature-tile per block;
    # a unit assigned to block b is emitted inside b's kt loop, and every
    # consumer of its qkT slice reads it in block b+1 or later
    slots = {(1, 0): 0, (1, 1): 1,
             (FT_AT + 1, 0): 1, (FT_AT + 1, 1): 2,
             (2, 0): 3, (2, 1): 4,
             (FT_AT + 2, 0): 5, (FT_AT + 2, 1): 6,
             (3, 0): 7, (3, 1): 8,
             (FT_AT + 3, 0): 9, (FT_AT + 3, 1): 10}
    fillers = {}
    for (f, pc2), bi in slots.items():
        fillers.setdefault(bi, []).append(
            lambda f=f, pc2=pc2: b1_mm(wts[f], f, pc2))
    # prologue: scores+exp of block 0, with the remaining projection chunks
    # and the v projection interleaved (all must precede block 0's PV)
    for kt in range(KT):
        if kt == 6:
            b1_mm(w4, FT_AT + 0, 1)   # k positions 1024:2048, before kt=8
        if kt == 10:
            b1_mm(w0, 0, 1)           # q positions 1024:2048, before qc=2
        emit_sc(E_bufs[0], 0, 0, kt)
        if kt % 2 == 1:
            b2_block(kt // 2)
    for i, (pp, qc) in enumerate(blocks):
        nxt = blocks[i + 1] if i + 1 < len(blocks) else None
        fl = fillers.get(i, [])
        pv = ps.tile([128, 2, PC], F32, tag="ps", name=f"pv{pp}_{qc}")
        for kt in range(KT):
            emit_pv(E_bufs[i % 2], pv, pp, kt)
            if nxt is not None:
                emit_sc(E_bufs[(i + 1) % 2], nxt[0], nxt[1], kt)
            if kt == 5 and len(fl) > 0:
                fl[0]()
            if kt == 11 and len(fl) > 1:
                fl[1]()
        emit_norm(pv, pp, qc)
        if pp == NH // 2 - 1:
            d_block(qc)
    return last_copy


def _get_nc():
    if "nc" not in _CACHE:
        _CACHE["nc"] = _build()
    return _CACHE["nc"]


def _make_in_maps(x, w_qkv, b_qkv, w_out):
    sel2 = np.zeros((2, 128), dtype=np.float32)
    sel2[0, 0:64] = 1.0
    sel2[1, 64:128] = 1.0
    in_maps = []
    for c in range(NCORES):
        b, half = divmod(c, 2)
        hs = half * QF
        in_maps.append({
            "xT": np.ascontiguousarray(x[b].T).astype(DT_NP),
            "wqk": np.concatenate([w_qkv[:, hs:hs + QF],
                                   w_qkv[:, D + hs:D + hs + QF]],
                                  axis=1).astype(DT_NP),
            "wv": np.ascontiguousarray(w_qkv[:, 2 * D + hs:2 * D + hs + QF]).astype(DT_NP),
            "bqk": np.concatenate([b_qkv[hs:hs + QF],
                                   b_qkv[D + hs:D + hs + QF]])[:, None].astype(np.float32),
            "wo": np.ascontiguousarray(w_out[hs:hs + QF, :]).astype(DT_NP),
            "sel2": sel2,
        })
    return in_maps


def kernel(x, w_qkv, b_qkv, w_out, b_out):
    x = np.asarray(x, dtype=np.float32)
    w_qkv = np.asarray(w_qkv, dtype=np.float32)
    b_qkv = np.asarray(b_qkv, dtype=np.float32)
    w_out = np.asarray(w_out, dtype=np.float32)
    b_out = np.asarray(b_out, dtype=np.float32)

    nc = _get_nc()
    in_maps = _make_in_maps(x, w_qkv, b_qkv, w_out)
    res = run_bass_kernel_spmd(nc, in_maps, list(range(NCORES)))
    _CACHE["last_results"] = res

    # host combine: out[b] = partial_A + partial_B + (b_out + bv @ w_out)
    const = b_out + b_qkv[2 * D:] @ w_out            # [D]
    out = np.empty((B, S, D), dtype=np.float32)
    for b in range(B):
        out[b] = (res.results[2 * b]["out_partial"]
                  + res.results[2 * b + 1]["out_partial"] + const)
    return out



# revision 29
# speedup vs baseline: 1.2419x; 1.2419x over previous
"""Trainium2 Bass kernel for multi-head self-attention.

Problem: B=4, S=2048, D=1024, H=16 heads (HD=64), fp32 I/O.
  qkv = x @ w_qkv + b_qkv ; attention(softmax(q k^T / 8) v) ; out @ w_out + b_out

Sharding over 8 NeuronCores: core c handles batch b=c//2 and heads
half=c%2 (8 heads each).  Each core computes a partial output
(its heads' contribution to out[b] @ w_out); the host sums the two
partials per batch and adds the constant bias terms.

Design notes (cost-model driven):
  - scores^T [keys, q] in fp16 (output-bound on the PE either way).
  - exp on ScalarE (the pacing engine: ~266 us of activation work).
  - PV computed TRANSPOSED: pvT[q, hd] = sum_k E[k, q] * v[k, hd]
    (lhsT = E tile, rhs = v).  Output free dim is only 65, so a full
    2048-key accumulation for 128 queries costs 16*65 rows instead of
    the 16*512 the untransposed form pays.  Row sums ride along as a
    fused ones-column in v; the softmax normalization becomes a
    per-partition scalar multiply on the DVE (no broadcast matmuls).
  - attn^T [q, f] is flipped back to attn [f, q] with DMA-engine
    transposes (XBAR), keeping the PE out of it.
  - QKV projections run as fp8e4 DoubleRow matmuls with an exact
    hi+lo split (x ~ x_hi + x_lo, w ~ w_hi + w_lo, dropping the
    lo*lo term): 3 half-rate passes = 75% of the fp16 cost, with
    ~0.1% relative error.  Scores/PV/out-proj stay fp16.
"""

import contextlib
import numpy as np
import ml_dtypes

import concourse.bacc as bacc
import concourse.tile as tile
from concourse.tile_rust import add_dep_helper
from concourse import mybir
from concourse.bass_utils import run_bass_kernel_spmd

B, S, D, H, HD = 4, 2048, 1024, 16, 64
NCORES = 8
NH = 8            # heads per core
QF = 512          # q features per core (= NH * HD), same for k and v
PC = 512          # position chunk (psum bank, fp32)
NPC = S // PC     # 4 position chunks
KT = S // 128     # 16 key-position tiles
QT = PC // 128    # 4 query tiles per position chunk
DC = D // 128     # 8 contraction chunks
DDC = DC // 2     # 4 double-contraction chunks (DoubleRow, K=256)
FT_QK = 8         # feature tiles of q+k (2*QF/128)
FT_AT = 4         # feature tiles of attn output (QF/128)

F32 = mybir.dt.float32
F16 = mybir.dt.float16
F8 = mybir.dt.float8e4
DR = mybir.MatmulPerfMode.DoubleRow

XS = 8.0          # host prescale for x (fp8 split)
WS = 128.0        # host prescale for w_qkv / w_v (fp8 split)
PSCALE = 1.0 / (XS * WS)   # undo on psum readout

_CACHE = {}


def _build(repeat=1):
    nc = bacc.Bacc("TRN2", target_bir_lowering=False, debug=False)

    # x / w_qk / w_v arrive host-permuted to the SBUF DoubleRow layout
    # (position-chunk-blocked for x) so every DMA lands as one contiguous
    # run per partition: [p, (pc,) ddc, t, *].
    xh = nc.dram_tensor("xh", [128, NPC, DDC, 2, PC], F8, kind="ExternalInput").ap()
    xl = nc.dram_tensor("xl", [128, NPC, DDC, 2, PC], F8, kind="ExternalInput").ap()
    xk0h = nc.dram_tensor("xk0h", [128, DDC, 2, 128], F8, kind="ExternalInput").ap()
    xk0l = nc.dram_tensor("xk0l", [128, DDC, 2, 128], F8, kind="ExternalInput").ap()
    wqkh = nc.dram_tensor("wqkh", [128, FT_QK, DDC, 2, 128], F8,
                          kind="ExternalInput").ap()
    wqkl = nc.dram_tensor("wqkl", [128, FT_QK, DDC, 2, 128], F8,
                          kind="ExternalInput").ap()
    wvh = nc.dram_tensor("wvh", [128, DDC, 2, QF], F8, kind="ExternalInput").ap()
    wvl = nc.dram_tensor("wvl", [128, DDC, 2, QF], F8, kind="ExternalInput").ap()
    bqk = nc.dram_tensor("bqk", [2 * QF, 1], F32, kind="ExternalInput").ap()
    wo = nc.dram_tensor("wo", [QF, D], F16, kind="ExternalInput").ap()
    out_d = nc.dram_tensor("out_partial", [S, D], F32, kind="ExternalOutput").ap()

    with tile.TileContext(nc) as tc:
        with contextlib.ExitStack() as ctx:
            with nc.allow_low_precision(reason="fp16/fp8 intermediates are intentional"):
                token = None
                for _ in range(repeat):
                    token = _emit(nc, tc, ctx, xh, xl, xk0h, xk0l,
                                  wqkh, wqkl, wvh, wvl,
                                  bqk, wo, out_d, token=token)
    nc.compile()
    return nc


def _emit(nc, tc, ctx, xh, xl, xk0h, xk0l, wqkh, wqkl, wvh, wvl,
          bqk, wo, out_d, token=None):
    with contextlib.ExitStack() as kctx:
        return _emit_inner(nc, tc, kctx, xh, xl, xk0h, xk0l, wqkh, wqkl,
                           wvh, wvl, bqk, wo, out_d, token)


def _emit_inner(nc, tc, ctx, xh, xl, xk0h, xk0l, wqkh, wqkl, wvh, wvl,
                bqk, wo, out_d, token=None):
    # ---- long-lived SBUF tensors -------------------------------------------
    keep = ctx.enter_context(tc.tile_pool(name="keep", bufs=1))
    qkT = keep.tile([128, FT_QK, S], F16, tag="qkT")             # 32 KB/p
    v_sb = keep.tile([128, KT, NH, HD + 1], F16, tag="v_sb")     # 16.3 KB/p
    attn = keep.tile([128, FT_AT, S], F16, tag="attn")           # 16 KB/p
    E_sb0 = keep.tile([128, KT, 2, PC], F16, tag="E_sb0")        # 32 KB/p
    E_sb1 = keep.tile([128, KT, 2, PC], F16, tag="E_sb1")        # 32 KB/p
    xt_h = keep.tile([128, NPC, DDC, 2, PC], F8, tag="xt_h")     # 16 KB/p
    xt_l = keep.tile([128, NPC, DDC, 2, PC], F8, tag="xt_l")     # 16 KB/p
    xk0_h = keep.tile([128, DDC, 2, 128], F8, tag="xk0_h")       # 1 KB/p
    xk0_l = keep.tile([128, DDC, 2, 128], F8, tag="xk0_l")       # 1 KB/p
    wv_h = keep.tile([128, DDC, 2, QF], F8, tag="wv_h")          # 4 KB/p
    wv_l = keep.tile([128, DDC, 2, QF], F8, tag="wv_l")          # 4 KB/p
    wo_t = keep.tile([128, FT_AT, D], F16, tag="wo_t")           # 8 KB/p
    bqk_t = keep.tile([128, FT_QK, 1], F32, tag="bqk")
    E_bufs = (E_sb0, E_sb1)

    wqk_pool = ctx.enter_context(tc.tile_pool(name="wqk_pool", bufs=2))
    at_pool = ctx.enter_context(tc.tile_pool(name="at_pool", bufs=2))
    rec_pool = ctx.enter_context(tc.tile_pool(name="rec_pool", bufs=2))
    stg = ctx.enter_context(tc.tile_pool(name="stg", bufs=2))
    ps = ctx.enter_context(tc.tile_pool(name="ps", bufs=1, space="PSUM"))

    def track(i):
        if token is not None:
            add_dep_helper(token.ins, i.ins, sync=True,
                           reason="serialize benchmark repeats")

    def emit_input_dmas():
        for pc in range(1, NPC):
            for hl, xsrc in ((xt_h, xh), (xt_l, xl)):
                track(nc.sync.dma_start(out=hl[:, pc, :, :, :],
                                        in_=xsrc[:, pc, :, :, :]))
        for hl, wsrc in ((wv_h, wvh), (wv_l, wvl)):
            track(nc.sync.dma_start(out=hl, in_=wsrc))
        for fc in range(FT_AT):
            track(nc.sync.dma_start(
                out=wo_t[:, fc, :], in_=wo[fc * 128:(fc + 1) * 128, :]))

    def x_dma_pc0():
        # the first key tile (0:128, host-duplicated contiguous) lands first
        # so the k projection for the first scores tile starts early
        track(nc.sync.dma_start(out=xk0_h, in_=xk0h))
        track(nc.sync.dma_start(out=xk0_l, in_=xk0l))
        track(nc.sync.dma_start(out=xt_h[:, 0, :, :, :], in_=xh[:, 0, :, :, :]))
        track(nc.sync.dma_start(out=xt_l[:, 0, :, :, :], in_=xl[:, 0, :, :, :]))

    def b1_dma(ft):
        """fetch one feature tile (hi+lo) of the qk weights."""
        wt_h = wqk_pool.tile([128, DDC, 2, 128], F8, tag="wqk_h", name=f"wqh{ft}")
        wt_l = wqk_pool.tile([128, DDC, 2, 128], F8, tag="wqk_l", name=f"wql{ft}")
        for hl, wsrc in ((wt_h, wqkh), (wt_l, wqkl)):
            track(nc.sync.dma_start(out=hl, in_=wsrc[:, ft, :, :, :]))
        return wt_h, wt_l

    def b1_mm(wt, ft, pc, lo=0, hi=PC):
        """qkT[f, s] = sum_d wqk[d, f] * xT[d, s] (+ bias), one pos chunk.

        3 DoubleRow passes (hi*hi, hi*lo, lo*hi), fp32 psum, one DVE
        op rescales by 2^-10 and adds the (prescaled) bias."""
        wt_h, wt_l = wt
        if (lo, hi) == (0, 128):
            xops = {id(xt_h): xk0_h, id(xt_l): xk0_l}
            xsl = lambda xop, dd: xops[id(xop)][:, dd, :, :]
        else:
            xsl = lambda xop, dd: xop[:, pc, dd, :, lo:hi]
        qp = ps.tile([128, hi - lo], F32, tag="misc", bufs=2,
                     name=f"qp{ft}_{pc}_{lo}")
        passes = ((wt_h, xt_h), (wt_h, xt_l), (wt_l, xt_h))
        n = 0
        for wop, xop in passes:
            for dd in range(DDC):
                nc.tensor.matmul(
                    qp, wop[:, dd, :, :], xsl(xop, dd),
                    start=(n == 0), stop=(n == 3 * DDC - 1), perf_mode=DR)
                n += 1
        nc.vector.tensor_scalar(
            out=qkT[:, ft, pc * PC + lo:pc * PC + hi], in0=qp,
            scalar1=bqk_t[:, ft, :],
            scalar2=PSCALE, op0=mybir.AluOpType.add, op1=mybir.AluOpType.mult)

    def b2_st(st):
        """v[s, f] natural layout (+ ones column), one 128-position tile."""
        pc, so = st // 4, (st % 4) * 128
        vp = ps.tile([128, QF], F32, tag="misc", bufs=2, name=f"vp{st}")
        passes = ((xt_h, wv_h), (xt_h, wv_l), (xt_l, wv_h))
        n = 0
        for xop, wop in passes:
            for dd in range(DDC):
                nc.tensor.matmul(
                    vp, xop[:, pc, dd, :, so:so + 128], wop[:, dd, :, :],
                    start=(n == 0), stop=(n == 3 * DDC - 1), perf_mode=DR)
                n += 1
        nc.vector.tensor_scalar(
            out=v_sb[:, st, :, 0:HD],
            in0=vp.rearrange("p (h d) -> p h d", h=NH),
            scalar1=PSCALE, scalar2=None, op0=mybir.AluOpType.mult)

    def emit_sc(E_sb, pp, qc, kt):
        """scores^T matmuls + exp for one kt tile of block (pp, qc)."""
        kft = FT_AT + pp
        qft = pp
        qs = slice(qc * PC, (qc + 1) * PC)
        ks = slice(kt * 128, (kt + 1) * 128)
        sc = ps.tile([128, 2, PC], F32, tag="sc", bufs=2, name=f"sc{pp}_{qc}_{kt}")
        nc.tensor.matmul(
            sc[:, 0, :], qkT[0:64, kft, ks], qkT[0:64, qft, qs],
            start=True, stop=True)
        nc.tensor.matmul(
            sc[:, 1, :], qkT[64:128, kft, ks], qkT[64:128, qft, qs],
            start=True, stop=True)
        nc.scalar.activation(
            out=E_sb[:, kt, :, :], in_=sc,
            func=mybir.ActivationFunctionType.Exp, scale=0.125)

    def emit_pvT(E_sb, pvt, pp, kt):
        """transposed PV for one kt tile: pvT[q, hd] += E[k, q] v[k, hd].

        PSUM accumulation groups are zero-region (2 KB bank) granular, so
        the four (qt, h) slices sharing a bank form ONE group: start on the
        bank's first matmul (zeroing the whole region), stop on its last."""
        for qt in range(QT):
            for h in range(2):
                first = kt == 0 and qt % 2 == 0 and h == 0
                last = kt == KT - 1 and qt % 2 == 1 and h == 1
                nc.tensor.matmul(
                    pvt[:, qt, h, 0:HD + 1],
                    E_sb[:, kt, h, qt * 128:(qt + 1) * 128],
                    v_sb[:, kt, 2 * pp + h, :],
                    start=first, stop=last, skip_group_check=True)

    last_copy = None

    def d_st(qc, j):
        """output projection for one position tile of q-chunk qc."""
        nonlocal last_copy
        st = qc * 4 + j
        ss = slice(st * 128, (st + 1) * 128)
        ot = stg.tile([128, D], F32, tag="ot")
        for n in range(2):
            op = ps.tile([128, PC], F32, tag="misc", bufs=2, name=f"op{st}_{n}")
            for fc in range(FT_AT):
                nc.tensor.matmul(
                    op, attn[:, fc, ss], wo_t[:, fc, n * PC:(n + 1) * PC],
                    start=(fc == 0), stop=(fc == FT_AT - 1))
            last_copy = nc.vector.tensor_copy(
                out=ot[:, n * PC:(n + 1) * PC], in_=op)
            nc.sync.dma_start(out=out_d[ss, n * PC:(n + 1) * PC],
                              in_=ot[:, n * PC:(n + 1) * PC])

    pq3 = keep.tile([128, 4, D], F16, tag="pq3")   # qc3 partial (fc 0..2)

    def d3_partial(j, n):
        """head pairs 0..2 of the last q-chunk's projection, done early."""
        st = 3 * 4 + j
        ss = slice(st * 128, (st + 1) * 128)
        op = ps.tile([128, PC], F32, tag="misc", bufs=2, name=f"pp3_{j}_{n}")
        for fc in range(FT_AT - 1):
            nc.tensor.matmul(
                op, attn[:, fc, ss], wo_t[:, fc, n * PC:(n + 1) * PC],
                start=(fc == 0), stop=(fc == FT_AT - 2))
        nc.vector.tensor_copy(out=pq3[:, j, n * PC:(n + 1) * PC], in_=op)

    def d3_final(j):
        """last head pair of the last q-chunk, fused with the partial."""
        nonlocal last_copy
        st = 3 * 4 + j
        ss = slice(st * 128, (st + 1) * 128)
        ot = stg.tile([128, D], F32, tag="ot")
        for n in range(2):
            op = ps.tile([128, PC], F32, tag="misc", bufs=2, name=f"fp3_{j}_{n}")
            nc.tensor.matmul(
                op, attn[:, FT_AT - 1, ss],
                wo_t[:, FT_AT - 1, n * PC:(n + 1) * PC], start=True, stop=True)
            last_copy = nc.vector.tensor_add(
                out=ot[:, n * PC:(n + 1) * PC], in0=op,
                in1=pq3[:, j, n * PC:(n + 1) * PC])
            nc.sync.dma_start(out=out_d[ss, n * PC:(n + 1) * PC],
                              in_=ot[:, n * PC:(n + 1) * PC])

    def emit_norm(pvt, pp, qc, on_act=False):
        """normalize with the fused row-sums and flip back to [f, s].  The
        final block's scalar multiplies run on the (by then idle) Act
        engine so the epilogue is not serialized behind DVE dispatch."""
        rec = rec_pool.tile([128, QT, 2, 1], F32, tag="rec")
        nc.vector.reciprocal(out=rec, in_=pvt[:, :, :, HD:HD + 1])
        at = at_pool.tile([128, QT, 2, HD], F16, tag="at")
        for qt in range(QT):
            for h in range(2):
                if on_act:
                    nc.scalar.activation(
                        out=at[:, qt, h, :], in_=pvt[:, qt, h, 0:HD],
                        func=mybir.ActivationFunctionType.Copy,
                        scale=rec[:, qt, h, :])
                else:
                    nc.vector.tensor_scalar_mul(
                        out=at[:, qt, h, :], in0=pvt[:, qt, h, 0:HD],
                        scalar1=rec[:, qt, h, :])
        nc.sync.dma_start_transpose(
            out=attn[:, pp, qc * PC:(qc + 1) * PC].rearrange(
                "p (qt q) -> p qt q", qt=QT),
            in_=at)

    # ---- emission schedule --------------------------------------------------
    # Tile directs dependencies by emission order, so every producer must be
    # emitted before its consumer.  The Act engine (exp) is the pacing
    # resource: block i's PV^T interleaves with block i+1's scores+exp so
    # the exp stream never starves.  QKV projection units are spread as
    # fillers ahead of their first consumer.
    nc.vector.memset(v_sb[:, :, :, HD:HD + 1], 1.0)
    track(nc.sync.dma_start(
        out=bqk_t, in_=bqk.rearrange("(ft p) o -> p ft o", p=128)))
    wts = {}
    wts[FT_AT] = b1_dma(FT_AT)     # k features of head pair 0
    wts[0] = b1_dma(0)             # q features of head pair 0
    x_dma_pc0()
    emit_input_dmas()
    b1_mm(wts[FT_AT], FT_AT, 0, 0, 128)   # k keys 0:128 (first sc tile)
    b1_mm(wts[0], 0, 0)                   # q positions 0:512
    for f in (FT_AT + 1, 1, FT_AT + 2, 2, FT_AT + 3, 3):
        wts[f] = b1_dma(f)

    # prologue: scores+exp of block 0; k chunks arrive just before needed,
    # q chunks for blocks 1-2 go early (they gate those blocks' scores),
    # v-projection units fill the remaining slack
    _prologue = {0: lambda: b1_mm(wts[FT_AT], FT_AT, 0, 128, PC),
                 1: lambda: b1_mm(wts[FT_AT], FT_AT, 1),
                 3: lambda: b1_mm(wts[0], 0, 1),
                 5: lambda: b1_mm(wts[FT_AT], FT_AT, 2),
                 7: lambda: b1_mm(wts[0], 0, 2),
                 9: lambda: b1_mm(wts[FT_AT], FT_AT, 3)}
    _b2_next = iter(range(KT))
    for kt in range(KT):
        emit_sc(E_bufs[0], 0, 0, kt)
        fn = _prologue.get(kt)
        if fn is not None:
            fn()
        else:
            b2_st(next(_b2_next))  # v st 0..9

    # filler units keyed (block_index, kt_slot).  Emission-order deadlines:
    # sc for block j is emitted during block j-1, so q(pp, qc) must be
    # emitted before block 4pp+qc-1 starts, and k(pp, *) before block
    # 4pp-1 starts.  v st feeds pvT(0, kt=st) emitted at block 1 slot st+2.
    fillers = {}

    def add_filler(bi, kt_slot, fn):
        fillers.setdefault(bi, {}).setdefault(kt_slot, []).append(fn)

    # remaining v projection: positions 1280:2048 early in block 0
    for u, st in enumerate(range(10, 16)):
        add_filler(0, 2 * u + 1, lambda st=st: b2_st(st))
    add_filler(0, 13, lambda: b1_mm(wts[0], 0, 3))
    for pp in range(1, 4):
        base = 4 * pp
        add_filler(base - 3, 5, lambda pp=pp: b1_mm(wts[FT_AT + pp], FT_AT + pp, 0))
        add_filler(base - 3, 9, lambda pp=pp: b1_mm(wts[FT_AT + pp], FT_AT + pp, 1))
        add_filler(base - 3, 13, lambda pp=pp: b1_mm(wts[FT_AT + pp], FT_AT + pp, 2))
        add_filler(base - 2, 3, lambda pp=pp: b1_mm(wts[FT_AT + pp], FT_AT + pp, 3))
        add_filler(base - 2, 11, lambda pp=pp: b1_mm(wts[pp], pp, 0))
        add_filler(base - 1, 11, lambda pp=pp: b1_mm(wts[pp], pp, 1))
        add_filler(base + 0, 11, lambda pp=pp: b1_mm(wts[pp], pp, 2))
        add_filler(base + 1, 11, lambda pp=pp: b1_mm(wts[pp], pp, 3))
    # qc3 partial projection (fc 0..2) once attn[:, 0..2, qc3] is final
    for u in range(6):
        add_filler(12, 2 * u + 1, lambda u=u: d3_partial(u // 2, u % 2))
    add_filler(13, 1, lambda: d3_partial(3, 0))
    add_filler(13, 5, lambda: d3_partial(3, 1))
    # qc0..qc2 output projections spread over the last blocks
    add_filler(13, 9, lambda: d_st(0, 0))
    add_filler(13, 13, lambda: d_st(0, 1))
    add_filler(14, 3, lambda: d_st(0, 2))
    add_filler(14, 9, lambda: d_st(0, 3))
    add_filler(14, 13, lambda: d_st(1, 0))
    add_filler(15, 1, lambda: d_st(1, 1))
    add_filler(15, 3, lambda: d_st(1, 2))
    add_filler(15, 5, lambda: d_st(1, 3))
    add_filler(15, 7, lambda: d_st(2, 0))
    add_filler(15, 9, lambda: d_st(2, 1))
    add_filler(15, 11, lambda: d_st(2, 2))
    add_filler(15, 13, lambda: d_st(2, 3))

    # main loop: block i emits sc+exp for block i+1.  The PV^T stream runs
    # one block late through block 7 (so the front-loaded v/qk projections
    # don't starve the exp stream), catches up during block 8 (which has
    # filler slack), and runs in-block from block 9 on (so the final
    # normalizations + projections drain before the epilogue).  pv units
    # trail their exp by >= 2 slots so the in-order PE queue never parks
    # on an exp that has not run yet.
    blocks = [(pp, qc) for pp in range(NH // 2) for qc in range(NPC)]
    pvts = {}

    def pvt_for(j):
        if j not in pvts:
            pp_j, qc_j = blocks[j]
            pvts[j] = ps.tile([128, QT, 2, 128], F32, tag="pvT", bufs=1,
                              name=f"pv{pp_j}_{qc_j}")
        return pvts[j]

    def pv_unit(j, jkt):
        emit_pvT(E_bufs[j % 2], pvt_for(j), blocks[j][0], jkt)

    for i, (pp, qc) in enumerate(blocks):
        nxt = blocks[i + 1] if i + 1 < len(blocks) else None
        fl = fillers.get(i, {})
        for kt in range(KT):
            if nxt is not None:
                emit_sc(E_bufs[(i + 1) % 2], nxt[0], nxt[1], kt)
            for fn in fl.get(kt, []):
                fn()
            if kt >= 2:
                pv_unit(i, kt - 2)
        pv_unit(i, KT - 2)
        pv_unit(i, KT - 1)
        emit_norm(pvt_for(i), pp, qc, on_act=(i == len(blocks) - 1))
    # epilogue: the last q-chunk's projection (head pair 3 + stored partial)
    for j in range(4):
        d3_final(j)
    return last_copy


def _get_nc():
    if "nc" not in _CACHE:
        _CACHE["nc"] = _build()
    return _CACHE["nc"]


def _split8(a, scale):
    """exact-ish hi+lo fp8e4 split of `a*scale` (both at the same scale)."""
    s = (a * scale).astype(np.float32)
    hi = s.astype(ml_dtypes.float8_e4m3)
    lo = (s - hi.astype(np.float32)).astype(ml_dtypes.float8_e4m3)
    return hi, lo


def _dr_x(a):
    """[D, S] -> pc-blocked DoubleRow layout [128, NPC, DDC, 2, PC]."""
    # d = dd*256 + t*128 + p
    return np.ascontiguousarray(
        a.reshape(DDC, 2, 128, NPC, PC).transpose(2, 3, 0, 1, 4))


def _dr_w(a, tiled):
    """[D, F] -> DoubleRow layout [128, ft, DDC, 2, 128] (tiled) or
    [128, DDC, 2, F] (flat)."""
    f = a.shape[1]
    if tiled:
        return np.ascontiguousarray(
            a.reshape(DDC, 2, 128, f // 128, 128).transpose(2, 3, 0, 1, 4))
    return np.ascontiguousarray(
        a.reshape(DDC, 2, 128, f).transpose(2, 0, 1, 3))


def _make_in_maps(x, w_qkv, b_qkv, w_out):
    in_maps = []
    for c in range(NCORES):
        b, half = divmod(c, 2)
        hs = half * QF
        xT = np.ascontiguousarray(x[b].T)
        wqk = np.concatenate([w_qkv[:, hs:hs + QF],
                              w_qkv[:, D + hs:D + hs + QF]], axis=1)
        wv = np.ascontiguousarray(w_qkv[:, 2 * D + hs:2 * D + hs + QF])
        xhi, xlo = _split8(xT, XS)
        whi, wlo = _split8(wqk, WS)
        vhi, vlo = _split8(wv, WS)
        xhi, xlo = _dr_x(xhi), _dr_x(xlo)
        in_maps.append({
            "xh": xhi, "xl": xlo,
            "xk0h": np.ascontiguousarray(xhi[:, 0, :, :, 0:128]),
            "xk0l": np.ascontiguousarray(xlo[:, 0, :, :, 0:128]),
            "wqkh": _dr_w(whi, True), "wqkl": _dr_w(wlo, True),
            "wvh": _dr_w(vhi, False), "wvl": _dr_w(vlo, False),
            "bqk": (np.concatenate([b_qkv[hs:hs + QF],
                                    b_qkv[D + hs:D + hs + QF]])[:, None]
                    / PSCALE).astype(np.float32),
            "wo": np.ascontiguousarray(w_out[hs:hs + QF, :]).astype(np.float16),
        })
    return in_maps


def kernel(x, w_qkv, b_qkv, w_out, b_out):
    x = np.asarray(x, dtype=np.float32)
    w_qkv = np.asarray(w_qkv, dtype=np.float32)
    b_qkv = np.asarray(b_qkv, dtype=np.float32)
    w_out = np.asarray(w_out, dtype=np.float32)
    b_out = np.asarray(b_out, dtype=np.float32)

    nc = _get_nc()
    in_maps = _make_in_maps(x, w_qkv, b_qkv, w_out)
    res = run_bass_kernel_spmd(nc, in_maps, list(range(NCORES)))
    _CACHE["last_results"] = res

    # host combine: out[b] = partial_A + partial_B + (b_out + bv @ w_out)
    const = b_out + b_qkv[2 * D:] @ w_out            # [D]
    out = np.empty((B, S, D), dtype=np.float32)
    for b in range(B):
        out[b] = (res.results[2 * b]["out_partial"]
                  + res.results[2 * b + 1]["out_partial"] + const)
    return out
